# revision 17
# baseline (speedup 1.0000x reference)
"""Butterworth bandpass (cascaded biquad IIR) Trainium2 kernel.

Problem: y = sosfilt(sos, x) over x[32, 64, 4096] fp32 -- 2048 independent
signals, 4 cascaded DF2T biquads, sequential over T=4096.

Strategy (exact block-parallel reformulation, no truncation):
  The cascade is a linear state-space system (A[8,8], B, C, D).  Split T into
  blocks of L=128, grouped in windows of R=4 blocks.  With s = state at the
  window entry, for block r of the window (all operators precomputed on host
  in float64 from the 24 sos coefficients):
      y_r = Th @ x_r + sum_{r'<r} (Z A_L^{r-r'-1} F) @ x_{r'} + (Z A_L^r) @ s
      s'  = A_L^R @ s + sum_r (A_L^{R-1-r} F) @ x_r
  On device everything is TensorE matmuls over [signal, time] tiles:
    - per block, transpose x[sig, time] -> xT[time, sig] on the PE;
    - one fused rhs table THW[128, 512] = [Th | ZF | ZA_LF | ZA_L^2F] turns
      conv + all intra-window cross-block corrections into a single
      accumulated matmul per source block (lhsT = xT_r, N = 512-128r);
    - entry-state corrections for all 4 blocks come from one matmul with
      rhs ZA[8, 512] (lhsT = s);
    - the state update accumulates in a [8, 256] psum.
  Matmul operands use dtype float32r (single-pass fp32 PE mode, 1 cyc/row at
  N>=256 vs 4 cyc/row for fp32 LOW_HIGH).  Conv outputs land directly in
  [signal, time] layout, so no output transpose is needed.  2048 signals are
  sharded 256 per NeuronCore (two groups of 128 output partitions).
"""

import numpy as np

import concourse.bass as bass
import concourse.tile as tile
from concourse import bacc
from concourse import mybir
from concourse.bass_utils import run_bass_kernel_spmd

FP32 = mybir.dt.float32
FP32R = mybir.dt.float32r

P = 128            # partition width == time-block length
T = 4096
NCORES = 8
NSIG = 2048        # 32*64 independent signals
SPC = NSIG // NCORES   # 256 signals per core
NST = 8            # state dim of the 4-biquad cascade
R = 4              # blocks per window
W = P * R          # 512 time steps per window (== DMA chunk)
NW = T // W        # 8 windows


# ----------------------------------------------------------------------------
# host-side: derive block-filter matrices from sos
# ----------------------------------------------------------------------------

def _build_system(sos):
    """Cascade of biquads (DF2T) -> single state space (A, B, C, D), float64."""
    sos = np.asarray(sos, dtype=np.float64)
    A = np.zeros((0, 0))
    B = np.zeros((0,))
    C = np.zeros((0,))
    D = 1.0
    for (b0, b1, b2, _one, a1, a2) in sos:
        As = np.array([[-a1, 1.0], [-a2, 0.0]])
        Bs = np.array([b1 - a1 * b0, b2 - a2 * b0])
        Cs = np.array([1.0, 0.0])
        Ds = b0
        n = A.shape[0]
        Anew = np.zeros((n + 2, n + 2))
        Anew[:n, :n] = A
        Anew[n:, :n] = np.outer(Bs, C)
        Anew[n:, n:] = As
        A = Anew
        B = np.concatenate([B, Bs * D])
        C = np.concatenate([Ds * C, Cs])
        D = Ds * D
    return A, B, C, D


def _balance(A, B, C):
    """Square-root balanced realization: both gramians become diagonal and
    equal, minimizing intermediate-magnitude disparity (important because the
    PE's float32r mode rounds products; unbalanced states reach |s|~650 and
    the rounding noise then dwarfs the O(1) output)."""
    P = np.outer(B, B)
    Ak = A.copy()
    for _ in range(64):
        P = P + Ak @ P @ Ak.T
        Ak = Ak @ Ak
    Q = np.outer(C, C)
    Ak = A.copy()
    for _ in range(64):
        Q = Q + Ak.T @ Q @ Ak
        Ak = Ak @ Ak
    Rc = np.linalg.cholesky(P + 1e-30 * np.eye(len(B)))
    M = Rc.T @ Q @ Rc
    lam, U = np.linalg.eigh(M)
    lam = np.maximum(lam, 1e-30)
    Tm = Rc @ U @ np.diag(lam ** -0.25)
    Ti = np.diag(lam ** 0.25) @ U.T @ np.linalg.inv(Rc)
    return Ti @ A @ Tm, Ti @ B, C @ Tm


def _build_matrices(sos):
    """Window-fused operator tables, all fp32 (fed to float32r device tiles).

    THW[128, 512]: cols [128d:128d+128] = Th (d=0) or (Z A_L^(d-1) F)^T (d>=1)
    ZA [8, 512]:   cols [128r:128r+128] = (Z A_L^r)^T
    FTR[128, 32]:  cols [8r:8r+8]       = ((A_L^(R-1-r)) F)^T
    A4T[8, 8]:     (A_L^R)^T
    """
    A, B, C, D = _build_system(sos)
    A, B, C = _balance(A, B, C)
    ns = A.shape[0]
    assert ns == NST

    h = np.zeros(P)
    h[0] = D
    An = np.eye(ns)
    for k in range(1, P):
        h[k] = C @ An @ B
        An = An @ A
    Th = np.zeros((P, P))
    for m in range(P):
        Th[m, m:] = h[: P - m]

    Z = np.zeros((P, ns))
    CAn = C.copy()
    for n in range(P):
        Z[n] = CAn
        CAn = CAn @ A

    F = np.zeros((ns, P))
    AmB = B.copy()
    for m in range(P - 1, -1, -1):
        F[:, m] = AmB
        AmB = A @ AmB

    AL = np.linalg.matrix_power(A, P)

    THW = np.zeros((P, R * P))
    THW[:, :P] = Th
    for d in range(1, R):
        THW[:, d * P:(d + 1) * P] = (Z @ np.linalg.matrix_power(AL, d - 1) @ F).T
    ZA = np.zeros((ns, R * P))
    for r in range(R):
        ZA[:, r * P:(r + 1) * P] = (Z @ np.linalg.matrix_power(AL, r)).T
    FTR = np.zeros((P, R * NST))
    for r in range(R):
        FTR[:, r * NST:(r + 1) * NST] = (np.linalg.matrix_power(AL, R - 1 - r) @ F).T
    A4T = np.linalg.matrix_power(AL, R).T

    f32 = lambda a: np.ascontiguousarray(a, dtype=np.float32)
    return f32(THW), f32(ZA), f32(FTR), f32(A4T)


# ----------------------------------------------------------------------------
# device kernel
# ----------------------------------------------------------------------------

def _build_nc():
    nc = bacc.Bacc("TRN2", target_bir_lowering=False)
    x_d = nc.dram_tensor("x", [SPC, T], FP32R, kind="ExternalInput").ap()
    thw_d = nc.dram_tensor("thw", [P, R * P], FP32R, kind="ExternalInput").ap()
    za_d = nc.dram_tensor("za", [NST, R * P], FP32R, kind="ExternalInput").ap()
    ftr_d = nc.dram_tensor("ftr", [P, R * NST], FP32R, kind="ExternalInput").ap()
    a4t_d = nc.dram_tensor("a4t", [NST, NST], FP32R, kind="ExternalInput").ap()
    id_d = nc.dram_tensor("ident", [P, P], FP32R, kind="ExternalInput").ap()
    s0_d = nc.dram_tensor("s0", [NST, 2 * P], FP32R, kind="ExternalInput").ap()
    y_d = nc.dram_tensor("y", [SPC, T], FP32, kind="ExternalOutput").ap()

    with tile.TileContext(nc) as tc:
        with (
            tc.tile_pool(name="consts", bufs=1) as consts,
            tc.tile_pool(name="xpool", bufs=3) as xpool,
            tc.tile_pool(name="ypool", bufs=3) as ypool,
            tc.tile_pool(name="xtpool", bufs=6) as xtpool,
            tc.tile_pool(name="spool", bufs=4) as spool,
            tc.tile_pool(name="pxt", bufs=2, space="PSUM") as pxt,
            tc.tile_pool(name="py", bufs=2, space="PSUM") as pyp,
            tc.tile_pool(name="ps", bufs=2, space="PSUM") as psp,
        ):
            ident = consts.tile([P, P], FP32R)
            nc.sync.dma_start(ident, id_d)
            thw_sb = consts.tile([P, R * P], FP32R)
            nc.sync.dma_start(thw_sb, thw_d)
            za_sb = consts.tile([NST, R * P], FP32R)
            nc.sync.dma_start(za_sb, za_d)
            ftr_sb = consts.tile([P, R * NST], FP32R)
            nc.sync.dma_start(ftr_sb, ftr_d)
            a4t_sb = consts.tile([NST, NST], FP32R)
            nc.sync.dma_start(a4t_sb, a4t_d)

            s_prev = spool.tile([NST, 2 * P], FP32R, tag="s")
            nc.sync.dma_start(s_prev, s0_d)

            # HAM warm-up: ~5us of dense bf16 matmuls while the first x DMAs
            # are in flight, so real matmuls start at K=8/8 (2.4 GHz) instead
            # of spending the first half of the kernel throttled at 1.2 GHz.
            warm_sb = consts.tile([P, W], mybir.dt.bfloat16)
            nc.vector.memset(warm_sb, 0.0)
            warm_ps = psp.tile([P, W], FP32, tag="warm", bufs=1, name="warm_ps")
            warm_i = [0]

            def keep_warm(n=1):
                # fp32r matmuls don't register as PE activity for the HAM
                # clock gate; sprinkle bf16 matmuls so the PE stays at 2.4GHz.
                for _ in range(n):
                    warm_i[0] += 1
                    nc.tensor.matmul(
                        warm_ps, warm_sb[:, :P], warm_sb, start=True, stop=True,
                        skip_group_check=True,
                    )

            keep_warm(12)

            for w in range(NW):
                x_sb = [
                    xpool.tile([P, W], FP32R, tag=f"x{g}", name=f"x_sb{g}")
                    for g in (0, 1)
                ]
                for g in (0, 1):
                    nc.sync.dma_start(
                        x_sb[g], x_d[g * P:(g + 1) * P, w * W:(w + 1) * W]
                    )
                y_sb = [
                    ypool.tile([P, W], FP32, tag=f"y{g}", name=f"y_sb{g}")
                    for g in (0, 1)
                ]

                # transpose the 4 blocks; xt_sb[r] = [time, sig(256)]
                xt_sb = []
                for r in range(R):
                    psum_t = pxt.tile([P, 2 * P], FP32R, tag="pxt", name=f"pst{r}")
                    for g in (0, 1):
                        nc.tensor.transpose(
                            psum_t[:, g * P:(g + 1) * P],
                            x_sb[g][:, r * P:(r + 1) * P],
                            ident,
                        )
                    xt = xtpool.tile([P, 2 * P], FP32R, tag="xt", name=f"xt{r}")
                    if r % 2 == 0:
                        nc.vector.tensor_copy(xt, psum_t)
                    else:
                        nc.scalar.copy(xt, psum_t)
                    xt_sb.append(xt)
                keep_warm(1)

                # y accumulation: per group one [128, 512] psum bank
                psum_y = [
                    pyp.tile([P, W], FP32, tag=f"py{g}", name=f"py{g}") for g in (0, 1)
                ]
                # x-dependent matmuls first (r=0 covers the full bank so it
                # can start the group); the s-dependent ZA/A4T matmuls go
                # LAST so the cross-window state chain has maximal slack.
                for g in (0, 1):
                    gs = slice(g * P, (g + 1) * P)
                    for r in range(R):
                        nc.tensor.matmul(
                            psum_y[g][:, r * P:],
                            xt_sb[r][:, gs],
                            thw_sb[:, : (R - r) * P],
                            start=(r == 0), stop=False,
                        )
                    nc.tensor.matmul(
                        psum_y[g], s_prev[:, gs], za_sb, start=False, stop=True,
                    )
                    keep_warm(1)

                # state update: psum_s[8, 256] over both groups
                psum_s = psp.tile([NST, 2 * P], FP32, tag="ps", bufs=1)
                for r in range(R):
                    nc.tensor.matmul(
                        psum_s, ftr_sb[:, r * NST:(r + 1) * NST], xt_sb[r],
                        start=(r == 0), stop=False,
                    )
                nc.tensor.matmul(psum_s, a4t_sb, s_prev, start=False, stop=True)
                s_next = spool.tile([NST, 2 * P], FP32R, tag="s")
                if w % 2 == 0:
                    nc.scalar.copy(s_next, psum_s)
                else:
                    nc.vector.tensor_copy(s_next, psum_s)
                s_prev = s_next

                # write back y and DMA out
                nc.vector.tensor_copy(y_sb[0], psum_y[0])
                nc.scalar.copy(y_sb[1], psum_y[1])
                for g in (0, 1):
                    nc.sync.dma_start(
                        y_d[g * P:(g + 1) * P, w * W:(w + 1) * W], y_sb[g]
                    )
    nc.compile()
    return nc


_NC_CACHE = None
LAST_RESULTS = None  # BassKernelResults of the most recent kernel() call


def _get_nc():
    global _NC_CACHE
    if _NC_CACHE is None:
        _NC_CACHE = _build_nc()
    return _NC_CACHE


def kernel(x: np.ndarray, sos: np.ndarray) -> np.ndarray:
    x = np.asarray(x)
    orig_shape = x.shape
    orig_dtype = x.dtype
    THW, ZA, FTR, A4T = _build_matrices(np.asarray(sos, dtype=np.float64))

    xf = np.ascontiguousarray(x.reshape(NSIG, T), dtype=np.float32)
    in_maps = [
        {
            "x": xf[c * SPC:(c + 1) * SPC],
            "thw": THW,
            "za": ZA,
            "ftr": FTR,
            "a4t": A4T,
            "ident": np.eye(P, dtype=np.float32),
            "s0": np.zeros((NST, 2 * P), dtype=np.float32),
        }
        for c in range(NCORES)
    ]
    nc = _get_nc()
    res = run_bass_kernel_spmd(nc, in_maps, core_ids=list(range(NCORES)))
    global LAST_RESULTS
    LAST_RESULTS = res
    y = np.concatenate([res.results[c]["y"] for c in range(NCORES)], axis=0)
    return y.reshape(orig_shape).astype(orig_dtype, copy=False)


# revision 18
# speedup vs baseline: 1.0126x; 1.0126x over previous
"""Butterworth bandpass (cascaded biquad IIR) Trainium2 kernel.

Problem: y = sosfilt(sos, x) over x[32, 64, 4096] fp32 -- 2048 independent
signals, 4 cascaded DF2T biquads, sequential over T=4096.

Strategy (exact block-parallel reformulation, no truncation):
  The cascade is a linear state-space system (A[8,8], B, C, D).  Split T into
  blocks of L=128, grouped in windows of R=4 blocks.  With s = state at the
  window entry, for block r of the window (all operators precomputed on host
  in float64 from the 24 sos coefficients):
      y_r = Th @ x_r + sum_{r'<r} (Z A_L^{r-r'-1} F) @ x_{r'} + (Z A_L^r) @ s
      s'  = A_L^R @ s + sum_r (A_L^{R-1-r} F) @ x_r
  On device everything is TensorE matmuls over [signal, time] tiles:
    - per block, transpose x[sig, time] -> xT[time, sig] on the PE;
    - one fused rhs table THW[128, 512] = [Th | ZF | ZA_LF | ZA_L^2F] turns
      conv + all intra-window cross-block corrections into a single
      accumulated matmul per source block (lhsT = xT_r, N = 512-128r);
    - entry-state corrections for all 4 blocks come from one matmul with
      rhs ZA[8, 512] (lhsT = s);
    - the state update accumulates in a [8, 256] psum.
  Matmul operands use dtype float32r (single-pass fp32 PE mode, 1 cyc/row at
  N>=256 vs 4 cyc/row for fp32 LOW_HIGH).  Conv outputs land directly in
  [signal, time] layout, so no output transpose is needed.  2048 signals are
  sharded 256 per NeuronCore (two groups of 128 output partitions).
"""

import numpy as np

import concourse.bass as bass
import concourse.tile as tile
from concourse import bacc
from concourse import mybir
from concourse.bass_utils import run_bass_kernel_spmd

FP32 = mybir.dt.float32
FP32R = mybir.dt.float32r

P = 128            # partition width == time-block length
T = 4096
NCORES = 8
NSIG = 2048        # 32*64 independent signals
SPC = NSIG // NCORES   # 256 signals per core
NST = 8            # state dim of the 4-biquad cascade
R = 4              # blocks per window
W = P * R          # 512 time steps per window (== DMA chunk)
NW = T // W        # 8 windows


# ----------------------------------------------------------------------------
# host-side: derive block-filter matrices from sos
# ----------------------------------------------------------------------------

def _build_system(sos):
    """Cascade of biquads (DF2T) -> single state space (A, B, C, D), float64."""
    sos = np.asarray(sos, dtype=np.float64)
    A = np.zeros((0, 0))
    B = np.zeros((0,))
    C = np.zeros((0,))
    D = 1.0
    for (b0, b1, b2, _one, a1, a2) in sos:
        As = np.array([[-a1, 1.0], [-a2, 0.0]])
        Bs = np.array([b1 - a1 * b0, b2 - a2 * b0])
        Cs = np.array([1.0, 0.0])
        Ds = b0
        n = A.shape[0]
        Anew = np.zeros((n + 2, n + 2))
        Anew[:n, :n] = A
        Anew[n:, :n] = np.outer(Bs, C)
        Anew[n:, n:] = As
        A = Anew
        B = np.concatenate([B, Bs * D])
        C = np.concatenate([Ds * C, Cs])
        D = Ds * D
    return A, B, C, D


def _balance(A, B, C):
    """Square-root balanced realization: both gramians become diagonal and
    equal, minimizing intermediate-magnitude disparity (important because the
    PE's float32r mode rounds products; unbalanced states reach |s|~650 and
    the rounding noise then dwarfs the O(1) output)."""
    P = np.outer(B, B)
    Ak = A.copy()
    for _ in range(64):
        P = P + Ak @ P @ Ak.T
        Ak = Ak @ Ak
    Q = np.outer(C, C)
    Ak = A.copy()
    for _ in range(64):
        Q = Q + Ak.T @ Q @ Ak
        Ak = Ak @ Ak
    Rc = np.linalg.cholesky(P + 1e-30 * np.eye(len(B)))
    M = Rc.T @ Q @ Rc
    lam, U = np.linalg.eigh(M)
    lam = np.maximum(lam, 1e-30)
    Tm = Rc @ U @ np.diag(lam ** -0.25)
    Ti = np.diag(lam ** 0.25) @ U.T @ np.linalg.inv(Rc)
    return Ti @ A @ Tm, Ti @ B, C @ Tm


def _build_matrices(sos):
    """Window-fused operator tables, all fp32 (fed to float32r device tiles).

    THW[128, 512]: cols [128d:128d+128] = Th (d=0) or (Z A_L^(d-1) F)^T (d>=1)
    ZA [8, 512]:   cols [128r:128r+128] = (Z A_L^r)^T
    FTR[128, 32]:  cols [8r:8r+8]       = ((A_L^(R-1-r)) F)^T
    A4T[8, 8]:     (A_L^R)^T
    """
    A, B, C, D = _build_system(sos)
    A, B, C = _balance(A, B, C)
    ns = A.shape[0]
    assert ns == NST

    h = np.zeros(P)
    h[0] = D
    An = np.eye(ns)
    for k in range(1, P):
        h[k] = C @ An @ B
        An = An @ A
    Th = np.zeros((P, P))
    for m in range(P):
        Th[m, m:] = h[: P - m]

    Z = np.zeros((P, ns))
    CAn = C.copy()
    for n in range(P):
        Z[n] = CAn
        CAn = CAn @ A

    F = np.zeros((ns, P))
    AmB = B.copy()
    for m in range(P - 1, -1, -1):
        F[:, m] = AmB
        AmB = A @ AmB

    AL = np.linalg.matrix_power(A, P)

    THW = np.zeros((P, R * P))
    THW[:, :P] = Th
    for d in range(1, R):
        THW[:, d * P:(d + 1) * P] = (Z @ np.linalg.matrix_power(AL, d - 1) @ F).T
    ZA = np.zeros((ns, R * P))
    for r in range(R):
        ZA[:, r * P:(r + 1) * P] = (Z @ np.linalg.matrix_power(AL, r)).T
    FTR = np.zeros((P, R * NST))
    for r in range(R):
        FTR[:, r * NST:(r + 1) * NST] = (np.linalg.matrix_power(AL, R - 1 - r) @ F).T
    A4T = np.linalg.matrix_power(AL, R).T

    f32 = lambda a: np.ascontiguousarray(a, dtype=np.float32)
    return f32(THW), f32(ZA), f32(FTR), f32(A4T)


# ----------------------------------------------------------------------------
# device kernel
# ----------------------------------------------------------------------------

def _build_nc():
    nc = bacc.Bacc("TRN2", target_bir_lowering=False)
    x_d = nc.dram_tensor("x", [SPC, T], FP32R, kind="ExternalInput").ap()
    thw_d = nc.dram_tensor("thw", [P, R * P], FP32R, kind="ExternalInput").ap()
    za_d = nc.dram_tensor("za", [NST, R * P], FP32R, kind="ExternalInput").ap()
    ftr_d = nc.dram_tensor("ftr", [P, R * NST], FP32R, kind="ExternalInput").ap()
    a4t_d = nc.dram_tensor("a4t", [NST, NST], FP32R, kind="ExternalInput").ap()
    id_d = nc.dram_tensor("ident", [P, P], FP32R, kind="ExternalInput").ap()
    s0_d = nc.dram_tensor("s0", [NST, 2 * P], FP32R, kind="ExternalInput").ap()
    y_d = nc.dram_tensor("y", [SPC, T], FP32, kind="ExternalOutput").ap()

    with tile.TileContext(nc) as tc:
        with (
            tc.tile_pool(name="consts", bufs=1) as consts,
            tc.tile_pool(name="xpool", bufs=3) as xpool,
            tc.tile_pool(name="ypool", bufs=3) as ypool,
            tc.tile_pool(name="xtpool", bufs=6) as xtpool,
            tc.tile_pool(name="spool", bufs=4) as spool,
            tc.tile_pool(name="pxt", bufs=2, space="PSUM") as pxt,
            tc.tile_pool(name="py", bufs=2, space="PSUM") as pyp,
            tc.tile_pool(name="ps", bufs=2, space="PSUM") as psp,
        ):
            ident = consts.tile([P, P], FP32R)
            nc.sync.dma_start(ident, id_d)
            thw_sb = consts.tile([P, R * P], FP32R)
            nc.sync.dma_start(thw_sb, thw_d)
            za_sb = consts.tile([NST, R * P], FP32R)
            nc.sync.dma_start(za_sb, za_d)
            ftr_sb = consts.tile([P, R * NST], FP32R)
            nc.sync.dma_start(ftr_sb, ftr_d)
            a4t_sb = consts.tile([NST, NST], FP32R)
            nc.sync.dma_start(a4t_sb, a4t_d)

            s_prev = spool.tile([NST, 2 * P], FP32R, tag="s")
            nc.sync.dma_start(s_prev, s0_d)

            for w in range(NW):
                x_sb = [
                    xpool.tile([P, W], FP32R, tag=f"x{g}", name=f"x_sb{g}")
                    for g in (0, 1)
                ]
                for g in (0, 1):
                    if w == 0:
                        # split the first window's loads so compute can begin
                        # as soon as the first 64KiB block lands
                        for r in range(R):
                            nc.sync.dma_start(
                                x_sb[g][:, r * P:(r + 1) * P],
                                x_d[g * P:(g + 1) * P, r * P:(r + 1) * P],
                            )
                    else:
                        nc.sync.dma_start(
                            x_sb[g], x_d[g * P:(g + 1) * P, w * W:(w + 1) * W]
                        )
                y_sb = [
                    ypool.tile([P, W], FP32, tag=f"y{g}", name=f"y_sb{g}")
                    for g in (0, 1)
                ]

                # transpose the 4 blocks; xt_sb[r] = [time, sig(256)]
                xt_sb = []
                for r in range(R):
                    psum_t = pxt.tile([P, 2 * P], FP32R, tag="pxt", name=f"pst{r}")
                    for g in (0, 1):
                        nc.tensor.transpose(
                            psum_t[:, g * P:(g + 1) * P],
                            x_sb[g][:, r * P:(r + 1) * P],
                            ident,
                        )
                    xt = xtpool.tile([P, 2 * P], FP32R, tag="xt", name=f"xt{r}")
                    if r % 2 == 0:
                        nc.vector.tensor_copy(xt, psum_t)
                    else:
                        nc.scalar.copy(xt, psum_t)
                    xt_sb.append(xt)

                # y accumulation: per group one [128, 512] psum bank
                psum_y = [
                    pyp.tile([P, W], FP32, tag=f"py{g}", name=f"py{g}") for g in (0, 1)
                ]
                # x-dependent matmuls first (r=0 covers the full bank so it
                # can start the group); the s-dependent ZA/A4T matmuls go
                # LAST so the cross-window state chain has maximal slack.
                for g in (0, 1):
                    gs = slice(g * P, (g + 1) * P)
                    for r in range(R):
                        nc.tensor.matmul(
                            psum_y[g][:, r * P:],
                            xt_sb[r][:, gs],
                            thw_sb[:, : (R - r) * P],
                            start=(r == 0), stop=False,
                        )
                    nc.tensor.matmul(
                        psum_y[g], s_prev[:, gs], za_sb, start=False, stop=True,
                    )

                # state update: psum_s[8, 256] over both groups
                psum_s = psp.tile([NST, 2 * P], FP32, tag="ps")
                for r in range(R):
                    nc.tensor.matmul(
                        psum_s, ftr_sb[:, r * NST:(r + 1) * NST], xt_sb[r],
                        start=(r == 0), stop=False,
                    )
                nc.tensor.matmul(psum_s, a4t_sb, s_prev, start=False, stop=True)
                s_next = spool.tile([NST, 2 * P], FP32R, tag="s")
                if w % 2 == 0:
                    nc.scalar.copy(s_next, psum_s)
                else:
                    nc.vector.tensor_copy(s_next, psum_s)
                s_prev = s_next

                # write back y and DMA out
                nc.vector.tensor_copy(y_sb[0], psum_y[0])
                nc.scalar.copy(y_sb[1], psum_y[1])
                for g in (0, 1):
                    nc.sync.dma_start(
                        y_d[g * P:(g + 1) * P, w * W:(w + 1) * W], y_sb[g]
                    )
    nc.compile()
    return nc


_NC_CACHE = None
LAST_RESULTS = None  # BassKernelResults of the most recent kernel() call


def _get_nc():
    global _NC_CACHE
    if _NC_CACHE is None:
        _NC_CACHE = _build_nc()
    return _NC_CACHE


def kernel(x: np.ndarray, sos: np.ndarray) -> np.ndarray:
    x = np.asarray(x)
    orig_shape = x.shape
    orig_dtype = x.dtype
    THW, ZA, FTR, A4T = _build_matrices(np.asarray(sos, dtype=np.float64))

    xf = np.ascontiguousarray(x.reshape(NSIG, T), dtype=np.float32)
    in_maps = [
        {
            "x": xf[c * SPC:(c + 1) * SPC],
            "thw": THW,
            "za": ZA,
            "ftr": FTR,
            "a4t": A4T,
            "ident": np.eye(P, dtype=np.float32),
            "s0": np.zeros((NST, 2 * P), dtype=np.float32),
        }
        for c in range(NCORES)
    ]
    nc = _get_nc()
    res = run_bass_kernel_spmd(nc, in_maps, core_ids=list(range(NCORES)))
    global LAST_RESULTS
    LAST_RESULTS = res
    y = np.concatenate([res.results[c]["y"] for c in range(NCORES)], axis=0)
    return y.reshape(orig_shape).astype(orig_dtype, copy=False)


# revision 19
# speedup vs baseline: 1.3314x; 1.3148x over previous
"""Butterworth bandpass (cascaded biquad IIR) Trainium2 kernel.

Problem: y = sosfilt(sos, x) over x[32, 64, 4096] fp32 -- 2048 independent
signals, 4 cascaded DF2T biquads, sequential over T=4096.

Strategy (exact block-parallel reformulation, no truncation):
  The cascade is a linear state-space system (A[8,8], B, C, D).  Split T into
  blocks of L=128, grouped in windows of R=4 blocks.  With s = state at the
  window entry, for block r of the window (all operators precomputed on host
  in float64 from the 24 sos coefficients):
      y_r = Th @ x_r + sum_{r'<r} (Z A_L^{r-r'-1} F) @ x_{r'} + (Z A_L^r) @ s
      s'  = A_L^R @ s + sum_r (A_L^{R-1-r} F) @ x_r
  On device everything is TensorE matmuls over [signal, time] tiles:
    - per block, transpose x[sig, time] -> xT[time, sig] on the PE;
    - one fused rhs table THW[128, 512] = [Th | ZF | ZA_LF | ZA_L^2F] turns
      conv + all intra-window cross-block corrections into a single
      accumulated matmul per source block (lhsT = xT_r, N = 512-128r);
    - entry-state corrections for all 4 blocks come from one matmul with
      rhs ZA[8, 512] (lhsT = s);
    - the state update accumulates in a [8, 256] psum.
  Matmul operands use dtype float32r (single-pass fp32 PE mode, 1 cyc/row at
  N>=256 vs 4 cyc/row for fp32 LOW_HIGH).  Conv outputs land directly in
  [signal, time] layout, so no output transpose is needed.  2048 signals are
  sharded 256 per NeuronCore (two groups of 128 output partitions).
"""

import numpy as np

import concourse.bass as bass
import concourse.tile as tile
from concourse import bacc
from concourse import mybir
from concourse.bass_utils import run_bass_kernel_spmd

FP32 = mybir.dt.float32
FP32R = mybir.dt.float32r

P = 128            # partition width == time-block length
T = 4096
NCORES = 8
NSIG = 2048        # 32*64 independent signals
SPC = NSIG // NCORES   # 256 signals per core
NST = 8            # state dim of the 4-biquad cascade
R = 4              # blocks per window
W = P * R          # 512 time steps per window (== DMA chunk)
NW = T // W        # 8 windows


# ----------------------------------------------------------------------------
# host-side: derive block-filter matrices from sos
# ----------------------------------------------------------------------------

def _build_system(sos):
    """Cascade of biquads (DF2T) -> single state space (A, B, C, D), float64."""
    sos = np.asarray(sos, dtype=np.float64)
    A = np.zeros((0, 0))
    B = np.zeros((0,))
    C = np.zeros((0,))
    D = 1.0
    for (b0, b1, b2, _one, a1, a2) in sos:
        As = np.array([[-a1, 1.0], [-a2, 0.0]])
        Bs = np.array([b1 - a1 * b0, b2 - a2 * b0])
        Cs = np.array([1.0, 0.0])
        Ds = b0
        n = A.shape[0]
        Anew = np.zeros((n + 2, n + 2))
        Anew[:n, :n] = A
        Anew[n:, :n] = np.outer(Bs, C)
        Anew[n:, n:] = As
        A = Anew
        B = np.concatenate([B, Bs * D])
        C = np.concatenate([Ds * C, Cs])
        D = Ds * D
    return A, B, C, D


def _balance(A, B, C):
    """Square-root balanced realization: both gramians become diagonal and
    equal, minimizing intermediate-magnitude disparity (important because the
    PE's float32r mode rounds products; unbalanced states reach |s|~650 and
    the rounding noise then dwarfs the O(1) output)."""
    P = np.outer(B, B)
    Ak = A.copy()
    for _ in range(64):
        P = P + Ak @ P @ Ak.T
        Ak = Ak @ Ak
    Q = np.outer(C, C)
    Ak = A.copy()
    for _ in range(64):
        Q = Q + Ak.T @ Q @ Ak
        Ak = Ak @ Ak
    Rc = np.linalg.cholesky(P + 1e-30 * np.eye(len(B)))
    M = Rc.T @ Q @ Rc
    lam, U = np.linalg.eigh(M)
    lam = np.maximum(lam, 1e-30)
    Tm = Rc @ U @ np.diag(lam ** -0.25)
    Ti = np.diag(lam ** 0.25) @ U.T @ np.linalg.inv(Rc)
    return Ti @ A @ Tm, Ti @ B, C @ Tm


def _build_matrices(sos):
    """Window-fused operator tables, all fp32 (fed to float32r device tiles).

    THW[128, 512]: cols [128d:128d+128] = Th (d=0) or (Z A_L^(d-1) F)^T (d>=1)
    ZA [8, 512]:   cols [128r:128r+128] = (Z A_L^r)^T
    FTR[128, 32]:  cols [8r:8r+8]       = ((A_L^(R-1-r)) F)^T
    A4T[8, 8]:     (A_L^R)^T
    """
    A, B, C, D = _build_system(sos)
    A, B, C = _balance(A, B, C)
    ns = A.shape[0]
    assert ns == NST

    h = np.zeros(P)
    h[0] = D
    An = np.eye(ns)
    for k in range(1, P):
        h[k] = C @ An @ B
        An = An @ A
    Th = np.zeros((P, P))
    for m in range(P):
        Th[m, m:] = h[: P - m]

    Z = np.zeros((P, ns))
    CAn = C.copy()
    for n in range(P):
        Z[n] = CAn
        CAn = CAn @ A

    F = np.zeros((ns, P))
    AmB = B.copy()
    for m in range(P - 1, -1, -1):
        F[:, m] = AmB
        AmB = A @ AmB

    AL = np.linalg.matrix_power(A, P)

    THW = np.zeros((P, R * P))
    THW[:, :P] = Th
    for d in range(1, R):
        THW[:, d * P:(d + 1) * P] = (Z @ np.linalg.matrix_power(AL, d - 1) @ F).T
    ZA = np.zeros((ns, R * P))
    for r in range(R):
        ZA[:, r * P:(r + 1) * P] = (Z @ np.linalg.matrix_power(AL, r)).T
    FTR = np.zeros((P, R * NST))
    for r in range(R):
        FTR[:, r * NST:(r + 1) * NST] = (np.linalg.matrix_power(AL, R - 1 - r) @ F).T
    A4T = np.linalg.matrix_power(AL, R).T

    f32 = lambda a: np.ascontiguousarray(a, dtype=np.float32)
    return f32(THW), f32(ZA), f32(FTR), f32(A4T)


# ----------------------------------------------------------------------------
# device kernel
# ----------------------------------------------------------------------------

def _build_nc():
    nc = bacc.Bacc("TRN2", target_bir_lowering=False)
    x_d = nc.dram_tensor("x", [SPC, T], FP32R, kind="ExternalInput").ap()
    thw_d = nc.dram_tensor("thw", [P, R * P], FP32R, kind="ExternalInput").ap()
    za_d = nc.dram_tensor("za", [NST, R * P], FP32R, kind="ExternalInput").ap()
    ftr_d = nc.dram_tensor("ftr", [P, R * NST], FP32R, kind="ExternalInput").ap()
    a4t_d = nc.dram_tensor("a4t", [NST, NST], FP32R, kind="ExternalInput").ap()
    id_d = nc.dram_tensor("ident", [P, P], FP32R, kind="ExternalInput").ap()
    s0_d = nc.dram_tensor("s0", [NST, 2 * P], FP32R, kind="ExternalInput").ap()
    y_d = nc.dram_tensor("y", [SPC, T], FP32, kind="ExternalOutput").ap()

    with tile.TileContext(nc) as tc:
        with (
            tc.tile_pool(name="consts", bufs=1) as consts,
            tc.tile_pool(name="xpool", bufs=3) as xpool,
            tc.tile_pool(name="ypool", bufs=3) as ypool,
            tc.tile_pool(name="xtpool", bufs=6) as xtpool,
            tc.tile_pool(name="spool", bufs=4) as spool,
            tc.tile_pool(name="pxt", bufs=2, space="PSUM") as pxt,
            tc.tile_pool(name="py", bufs=2, space="PSUM") as pyp,
            tc.tile_pool(name="ps", bufs=2, space="PSUM") as psp,
        ):
            ident = consts.tile([P, P], FP32R)
            nc.sync.dma_start(ident, id_d)
            thw_sb = consts.tile([P, R * P], FP32R)
            nc.sync.dma_start(thw_sb, thw_d)
            za_sb = consts.tile([NST, R * P], FP32R)
            nc.sync.dma_start(za_sb, za_d)
            ftr_sb = consts.tile([P, R * NST], FP32R)
            nc.sync.dma_start(ftr_sb, ftr_d)
            a4t_sb = consts.tile([NST, NST], FP32R)
            nc.sync.dma_start(a4t_sb, a4t_d)

            s_prev = spool.tile([NST, 2 * P], FP32R, tag="s")
            nc.sync.dma_start(s_prev, s0_d)

            for w in range(NW):
                x_sb = [
                    xpool.tile([P, W], FP32R, tag=f"x{g}", name=f"x_sb{g}")
                    for g in (0, 1)
                ]
                for g in (0, 1):
                    nc.sync.dma_start(
                        x_sb[g], x_d[g * P:(g + 1) * P, w * W:(w + 1) * W]
                    )
                y_sb = [
                    ypool.tile([P, W], FP32, tag=f"y{g}", name=f"y_sb{g}")
                    for g in (0, 1)
                ]

                # transpose the 4 blocks; xt_sb[r] = [time, sig(256)]
                xt_sb = []
                for r in range(R):
                    psum_t = pxt.tile([P, 2 * P], FP32R, tag="pxt", name=f"pst{r}")
                    for g in (0, 1):
                        nc.tensor.transpose(
                            psum_t[:, g * P:(g + 1) * P],
                            x_sb[g][:, r * P:(r + 1) * P],
                            ident,
                        )
                    xt = xtpool.tile([P, 2 * P], FP32R, tag="xt", name=f"xt{r}")
                    if r % 2 == 0:
                        nc.vector.tensor_copy(xt, psum_t)
                    else:
                        nc.scalar.copy(xt, psum_t)
                    xt_sb.append(xt)

                # y accumulation: per group one [128, 512] psum bank
                psum_y = [
                    pyp.tile([P, W], FP32, tag=f"py{g}", name=f"py{g}") for g in (0, 1)
                ]
                for g in (0, 1):
                    gs = slice(g * P, (g + 1) * P)
                    nc.tensor.matmul(
                        psum_y[g], s_prev[:, gs], za_sb, start=True, stop=False,
                    )
                    for r in range(R):
                        nc.tensor.matmul(
                            psum_y[g][:, r * P:],
                            xt_sb[r][:, gs],
                            thw_sb[:, : (R - r) * P],
                            start=False, stop=(r == R - 1),
                        )

                # state update: psum_s[8, 256] over both groups
                psum_s = psp.tile([NST, 2 * P], FP32, tag="ps")
                nc.tensor.matmul(psum_s, a4t_sb, s_prev, start=True, stop=False)
                for r in range(R):
                    nc.tensor.matmul(
                        psum_s, ftr_sb[:, r * NST:(r + 1) * NST], xt_sb[r],
                        start=False, stop=(r == R - 1),
                    )
                s_next = spool.tile([NST, 2 * P], FP32R, tag="s")
                if w % 2 == 0:
                    nc.scalar.copy(s_next, psum_s)
                else:
                    nc.vector.tensor_copy(s_next, psum_s)
                s_prev = s_next

                # write back y and DMA out
                nc.vector.tensor_copy(y_sb[0], psum_y[0])
                nc.scalar.copy(y_sb[1], psum_y[1])
                for g in (0, 1):
                    nc.sync.dma_start(
                        y_d[g * P:(g + 1) * P, w * W:(w + 1) * W], y_sb[g]
                    )
    nc.compile()
    return nc


_NC_CACHE = None
LAST_RESULTS = None  # BassKernelResults of the most recent kernel() call


def _get_nc():
    global _NC_CACHE
    if _NC_CACHE is None:
        _NC_CACHE = _build_nc()
    return _NC_CACHE


def kernel(x: np.ndarray, sos: np.ndarray) -> np.ndarray:
    x = np.asarray(x)
    orig_shape = x.shape
    orig_dtype = x.dtype
    THW, ZA, FTR, A4T = _build_matrices(np.asarray(sos, dtype=np.float64))

    xf = np.ascontiguousarray(x.reshape(NSIG, T), dtype=np.float32)
    in_maps = [
        {
            "x": xf[c * SPC:(c + 1) * SPC],
            "thw": THW,
            "za": ZA,
            "ftr": FTR,
            "a4t": A4T,
            "ident": np.eye(P, dtype=np.float32),
            "s0": np.zeros((NST, 2 * P), dtype=np.float32),
        }
        for c in range(NCORES)
    ]
    nc = _get_nc()
    res = run_bass_kernel_spmd(nc, in_maps, core_ids=list(range(NCORES)))
    global LAST_RESULTS
    LAST_RESULTS = res
    y = np.concatenate([res.results[c]["y"] for c in range(NCORES)], axis=0)
    return y.reshape(orig_shape).astype(orig_dtype, copy=False)
